# revision 59
# baseline (speedup 1.0000x reference)
"""ANI2x-core MoE routing kernel for 8 Trainium2 NeuronCores.

Strategy:
  - Host routes atoms by species (stable argsort) so each expert MLP runs only
    on its own atoms (7x less compute than the masked reference).
  - Data-parallel across 8 cores: each species' atoms are split evenly over the
    cores; every core holds all 7 expert weight sets.
  - aev is pre-transposed on host to [1024, R] (feature-major, zero padded from
    1008, with a constant-1 row at 1008 for the layer-1 bias) and cast to bf16
    so TensorEngine consumes [K-partition, atom] tiles with no transpose.
  - All biases are folded into the matmuls: layer 1 via the xt ones-row;
    layers 2/3 via a ones-row in the spare partition of the last K-chunk (or a
    K=1 ones-matmul when the chunk is full); layer 4 via the ScalarE Copy bias.
  - CELU(0.1) is computed exactly via the identity
        10*celu(u) + 1 = min(exp(y), 1) + relu(y),   y = 10*u = psum
    with the x10 / +1 affine folded into the next layer's weights on host.
    Per layer-chunk: ScalarE Exp from PSUM, DVE tensor_scalar min (bf16 4x),
    then one DVE scalar_tensor_tensor max(psum,0)+m that fuses relu and add.
  - 4-stage software pipeline (L1(t) | L2(t-1) | L3(t-2) | L4(t-3), tails
    emitted first) keeps TensorE dense so the HAM clock stays at 2.4 GHz;
    per-chunk single-bank PSUM tiles (6+2 slots) avoid accumulation-group
    stalls; whole-segment K-chunk DMAs give 2.3KB descriptor lines.
"""

import numpy as np
import ml_dtypes
from contextlib import ExitStack

import concourse.bass as bass
import concourse.mybir as mybir
import concourse.tile as tile
from concourse import bacc
from concourse.bass_utils import run_bass_kernel_spmd

BF16NP = ml_dtypes.bfloat16
BF16 = mybir.dt.bfloat16
F32 = mybir.dt.float32

NCORES = 8
AEV = 1008
AEV_PAD = 1024
NSPEC = 7
LAYERS = [
    [256, 192, 160],  # H
    [224, 192, 160],  # C
    [192, 160, 128],  # O
    [192, 160, 128],  # N
    [160, 128, 96],   # S
    [160, 128, 96],   # F
    [160, 128, 96],   # Cl
]
A_MAX = 512          # atoms per supertile
SEG_ALIGN = 8

LAST_EXEC_NS = None
LAST_TRACE = None
LAST_RESULTS = None


def _fchunks(F):
    return [(i, min(128, F - i)) for i in range(0, F, 128)]


def _pad(F):
    """Pad feature widths in (128, 256) up to 256 (zero weight columns).

    Padded activations compute to exactly 1.0 (y=0 -> min(e^0,1)+relu(0)=1),
    so a pad row doubles as the ones-row for the next layer's folded bias."""
    return 256 if 128 < F < 256 else F


def _kinfo(K_real, K_pad):
    """K-chunk plan over the padded contraction dim.

    Returns (chunks, bias_mm, bias_row): chunks = [(lo, kw)]; if bias_row is
    not None the bias lives in weight row bias_row of the last chunk (a pad
    row of the input, which equals 1). bias_mm means an extra K=1 ones-matmul
    carries the bias instead.
    """
    chunks = [(lo, min(128, K_pad - lo)) for lo in range(0, K_pad, 128)]
    if K_pad > K_real:
        return chunks, False, K_real - 128 * (len(chunks) - 1)
    return chunks, True, None


def _pack_weights(params):
    """Pack all transformed weights (+ folded biases) into one bf16 array.

    Layout per species: [W1 k-blocks (8 x F1 cols)] [W2 k-blocks (F2 cols
    each, bias row folded; +1 extra F2-col bias block if bias_mm)] [W3 ...]
    [W4 (n4 cols)].
    """
    cols = []
    meta = []
    for s in range(NSPEC):
        F1, F2, F3 = LAYERS[s]
        P1, P2, P3 = _pad(F1), _pad(F2), _pad(F3)
        k2, bm2, br2 = _kinfo(F1, P1)
        k3, bm3, br3 = _kinfo(F2, P2)
        k4 = [(lo, min(128, P3 - lo)) for lo in range(0, P3, 128)]
        n4 = len(k4)
        c1 = 8 * P1
        c2 = (len(k2) + (1 if bm2 else 0)) * P2
        c3 = (len(k3) + (1 if bm3 else 0)) * P3
        meta.append(dict(P=(P1, P2, P3), k2=k2, bm2=bm2, br2=br2,
                         k3=k3, bm3=bm3, br3=br3, k4=k4, n4=n4,
                         off1=sum(cols), off2=sum(cols) + c1,
                         off3=sum(cols) + c1 + c2,
                         off4=sum(cols) + c1 + c2 + c3,
                         ncols=c1 + c2 + c3 + n4))
        cols.append(c1 + c2 + c3 + n4)

    Wp = np.zeros((128, sum(cols)), BF16NP)
    b4f = []

    for s in range(NSPEC):
        F1, F2, F3 = LAYERS[s]
        m = meta[s]
        P1, P2, P3 = m["P"]
        net = params[s]
        W = [np.asarray(w, np.float32) for w, _ in net]
        B = [np.asarray(b, np.float32) for _, b in net]

        # L1: weights and bias scaled by 10; ones-row at aev row 1008
        W1h = (10.0 * W[0]).astype(BF16NP)
        bh1 = (10.0 * B[0]).astype(BF16NP)
        for k in range(8):
            r0 = k * 128
            rw = min(128, AEV - r0)
            blk = Wp[:, m["off1"] + k * P1: m["off1"] + k * P1 + F1]
            if rw > 0:
                blk[0:rw] = W1h[r0:r0 + rw]
            if r0 <= AEV < r0 + 128:
                blk[AEV - r0] = bh1

        # L2/L3: unscaled bf16 weights; bias = 10*b - colsum(bf16 W),
        # placed in the pad/ones row of the last K chunk (or an extra block)
        for (Wf, Bf, off, Fr, Fp, kl, bm, br) in (
                (W[1], B[1], m["off2"], F2, P2, m["k2"], m["bm2"], m["br2"]),
                (W[2], B[2], m["off3"], F3, P3, m["k3"], m["bm3"], m["br3"])):
            Wh = Wf.astype(BF16NP)
            bh = (10.0 * Bf - Wh.astype(np.float32).sum(axis=0)).astype(BF16NP)
            for j, (lo, kw) in enumerate(kl):
                blk = Wp[:, off + j * Fp: off + j * Fp + Fr]
                n = max(0, min(kw, Wh.shape[0] - lo))
                if n > 0:
                    blk[0:n] = Wh[lo:lo + n]
                if br is not None and j == len(kl) - 1:
                    blk[br] = bh
            if bm:
                Wp[0, off + len(kl) * Fp: off + len(kl) * Fp + Fr] = bh

        # L4: unscaled bf16; bias correction as float (via ScalarE Copy)
        W4h = W[3].astype(BF16NP)
        b4f.append(float(B[3][0] - 0.1 * W4h.astype(np.float32).sum(axis=0)[0]))
        for j, (lo, kw) in enumerate(m["k4"]):
            n = max(0, min(kw, F3 - lo))
            if n > 0:
                Wp[0:n, m["off4"] + j] = W4h[lo:lo + n, 0]

    return dict(Wp=Wp, b4f=b4f, meta=meta)


def _build_graph(R, seg_off, seg_len, packed):
    """Build the SPMD single-core Bass graph (same on all 8 cores)."""
    nc = bacc.Bacc()

    Wp = packed["Wp"]
    meta = packed["meta"]
    b4f = packed["b4f"]

    xtd = nc.declare_dram_parameter("xt", [AEV_PAD, R], BF16, isOutput=False)
    wd = nc.declare_dram_parameter("w", list(Wp.shape), BF16, isOutput=False)
    outd = nc.declare_dram_parameter("out", [1, R], F32, isOutput=True)

    amax = mybir.AluOpType.max
    amin = mybir.AluOpType.min
    add = mybir.AluOpType.add
    Exp = mybir.ActivationFunctionType.Exp
    Copy = mybir.ActivationFunctionType.Copy

    Lmax = max(seg_len)

    with ExitStack() as ctx:
        tc = ctx.enter_context(tile.TileContext(nc))
        wpool = ctx.enter_context(tc.tile_pool(name="weights", bufs=1))
        xpool = ctx.enter_context(tc.tile_pool(name="x", bufs=16))
        hpool = ctx.enter_context(tc.tile_pool(name="h", bufs=12))
        epool = ctx.enter_context(tc.tile_pool(name="er", bufs=12))
        ypool = ctx.enter_context(tc.tile_pool(name="y", bufs=3))
        pspool = ctx.enter_context(tc.tile_pool(name="ps", bufs=7, space="PSUM"))
        ps4pool = ctx.enter_context(tc.tile_pool(name="ps4", bufs=1, space="PSUM"))

        ones = wpool.tile([1, Lmax], BF16, tag="ones")
        nc.gpsimd.memset(ones, 1.0)

        def epi_re(ps, fw, A):
            """exp + clamp from one chunk's psum: m = min(exp(ps), 1)."""
            e = epool.tile([128, A_MAX], BF16, tag="e")
            m = epool.tile([128, A_MAX], BF16, tag="m")
            sl = (slice(0, fw), slice(0, A))
            nc.scalar.activation(e[sl], ps[sl], Exp, bias=0.0, scale=1.0)
            nc.vector.tensor_scalar_min(m[sl], e[sl], 1.0)
            return ps, m

        def epi_fuse(pairs, fchn, A, htile):
            for ci, (f0, fw) in enumerate(fchn):
                ps, m = pairs[ci]
                nc.vector.scalar_tensor_tensor(
                    htile[0:fw, ci * A:ci * A + A], ps[0:fw, 0:A], 0.0,
                    m[0:fw, 0:A], amax, add)

        def hidden_layer(hin, klist, bias_mm, woff, Fp, wt, A):
            fch = _fchunks(Fp)
            nmm = len(klist) + (1 if bias_mm else 0)
            pairs = []
            for ci, (f0, fw) in enumerate(fch):
                ps = pspool.tile([128, A_MAX], F32, tag="ps")
                for n0 in range(0, A, 512):
                    nw = min(512, A - n0)
                    mi = 0
                    for j, (lo, kw) in enumerate(klist):
                        nc.tensor.matmul(
                            ps[0:fw, n0:n0 + nw],
                            wt[0:kw, woff + j * Fp + f0: woff + j * Fp + f0 + fw],
                            hin[0:kw, j * A + n0: j * A + n0 + nw],
                            start=(mi == 0), stop=(mi == nmm - 1))
                        mi += 1
                    if bias_mm:
                        nc.tensor.matmul(
                            ps[0:fw, n0:n0 + nw],
                            wt[0:1, woff + len(klist) * Fp + f0:
                               woff + len(klist) * Fp + f0 + fw],
                            ones[0:1, n0:n0 + nw],
                            start=False, stop=True)
                pairs.append(epi_re(ps, fw, A))
            hout = hpool.tile([128, 2 * A_MAX], BF16, tag="h")
            epi_fuse(pairs, fch, A, hout)
            return hout

        def emit_l2(st):
            s, t0, A, h1, wt, y_seg, last = st
            m = meta[s]
            h2 = hidden_layer(h1, m["k2"], m["bm2"], m["off2"] - m["off1"],
                              m["P"][1], wt, A)
            return (s, t0, A, h2, wt, y_seg, last)

        def emit_l3(st):
            s, t0, A, h2, wt, y_seg, last = st
            m = meta[s]
            h3 = hidden_layer(h2, m["k3"], m["bm3"], m["off3"] - m["off1"],
                              m["P"][2], wt, A)
            return (s, t0, A, h3, wt, y_seg, last)

        def emit_l4(st):
            s, t0, A, h3, wt, y_seg, last = st
            m = meta[s]
            kch4 = m["k4"]
            ps4 = ps4pool.tile([1, A_MAX], F32, tag="ps4")
            for n0 in range(0, A, 512):
                nw = min(512, A - n0)
                for j, (lo, kw) in enumerate(kch4):
                    nc.tensor.matmul(
                        ps4[0:1, n0:n0 + nw],
                        wt[0:kw, m["off4"] - m["off1"] + j: m["off4"] - m["off1"] + j + 1],
                        h3[0:kw, j * A + n0: j * A + n0 + nw],
                        start=(j == 0), stop=(j == len(kch4) - 1))
            nc.scalar.activation(y_seg[0:1, t0:t0 + A], ps4[0:1, 0:A], Copy,
                                 bias=b4f[s], scale=0.1)
            if last:
                L = seg_len[s]
                nc.sync.dma_start(out=outd[0:1, seg_off[s]:seg_off[s] + L],
                                  in_=y_seg[0:1, 0:L])

        def emit_l1(s, wt, xk, y_seg, t0, L, A):
            P1 = meta[s]["P"][0]
            fch1 = _fchunks(P1)
            pairs = []
            for ci, (f0, fw) in enumerate(fch1):
                ps = pspool.tile([128, A_MAX], F32, tag="ps")
                for n0 in range(0, A, 512):
                    nw = min(512, A - n0)
                    for k in range(8):
                        nc.tensor.matmul(
                            ps[0:fw, n0:n0 + nw],
                            wt[:, k * P1 + f0: k * P1 + f0 + fw],
                            xk[k][:, t0 + n0: t0 + n0 + nw],
                            start=(k == 0), stop=(k == 7))
                pairs.append(epi_re(ps, fw, A))
            h1 = hpool.tile([128, 2 * A_MAX], BF16, tag="h")
            epi_fuse(pairs, fch1, A, h1)
            return (s, t0, A, h1, wt, y_seg, t0 + A >= L)

        # 3-stage software pipeline over all supertiles, tails emitted first
        work = []   # (s, t0, A) in order
        for s in range(NSPEC):
            t0 = 0
            first = A_MAX
            while t0 < seg_len[s]:
                A = min(first if t0 == 0 else A_MAX, seg_len[s] - t0)
                work.append((s, t0, A))
                t0 += A

        p1 = p2 = p3 = None   # stage outputs awaiting next stage
        cur_s = -1
        wt = xk = y_seg = None
        for (s, t0, Awk) in work:
            if s != cur_s:
                # new species: segment DMAs (first supertile's columns first)
                m = meta[s]
                L = seg_len[s]
                xk = []
                c0 = min(A_MAX, L) if s == 0 else L
                for k in range(8):
                    xt_k = xpool.tile([128, Lmax], BF16, tag="xk")
                    eng = nc.sync if (s > 0 or k % 2 == 0) else nc.scalar
                    eng.dma_start(
                        out=xt_k[:, 0:c0],
                        in_=xtd[k * 128:(k + 1) * 128, seg_off[s]:seg_off[s] + c0])
                    xk.append(xt_k)
                wt = wpool.tile([128, m["ncols"]], BF16, tag=f"w_{s}")
                if s == 0:
                    # layer-1 block first (gates the first matmul), rest second
                    c1w = 8 * meta[0]["P"][0]
                    nc.scalar.dma_start(out=wt[:, 0:c1w],
                                        in_=wd[:, m["off1"]:m["off1"] + c1w])
                    nc.sync.dma_start(
                        out=wt[:, c1w:m["ncols"]],
                        in_=wd[:, m["off1"] + c1w:m["off1"] + m["ncols"]])
                else:
                    nc.sync.dma_start(out=wt,
                                      in_=wd[:, m["off1"]:m["off1"] + m["ncols"]])
                if c0 < L:
                    for k in range(8):
                        nc.sync.dma_start(
                            out=xk[k][:, c0:L],
                            in_=xtd[k * 128:(k + 1) * 128,
                                    seg_off[s] + c0:seg_off[s] + L])
                y_seg = ypool.tile([1, Lmax], F32, tag="y")
                cur_s = s
            if p3 is not None:
                emit_l4(p3)
            p3 = emit_l3(p2) if p2 is not None else None
            p2 = emit_l2(p1) if p1 is not None else None
            p1 = emit_l1(s, wt, xk, y_seg, t0, seg_len[s], Awk)
        if p3 is not None:
            emit_l4(p3)
        if p2 is not None:
            emit_l4(emit_l3(p2))
        if p1 is not None:
            emit_l4(emit_l3(emit_l2(p1)))

    nc.finalize()
    return nc


def _ensure_ntff_hook():
    """Provide antenv.axon_hooks (missing in this image) so trace=True works."""
    try:
        from antenv.axon_hooks import get_axon_ntff_profile_hook
        return get_axon_ntff_profile_hook() is not None
    except ImportError:
        pass
    try:
        import types, sys
        import antenv
        from trn_agent_boot.trn_boot import _ntff_profile_via_ctypes
        hook = _ntff_profile_via_ctypes("/opt/axon/libaxon_pjrt.so")
        if hook is None:
            return False
        mod = types.ModuleType("antenv.axon_hooks")
        state = {"hook": hook}
        mod.get_axon_ntff_profile_hook = lambda: state["hook"]
        mod.set_axon_ntff_profile_hook = lambda h: state.__setitem__("hook", h)
        sys.modules["antenv.axon_hooks"] = mod
        antenv.axon_hooks = mod
        import concourse.bass_utils as bu
        bu.upload_artifacts = lambda tmpdir: f"local://{tmpdir}"
        return True
    except Exception:
        import traceback
        traceback.print_exc()
        return False


def kernel(species, aev, params):
    global LAST_EXEC_NS, LAST_TRACE, LAST_RESULTS
    species = np.asarray(species, np.int32)
    aev = np.asarray(aev, np.float32)
    n_atoms = species.shape[0]

    order = np.argsort(species, kind="stable")
    counts = np.bincount(species, minlength=NSPEC)

    starts = np.concatenate([[0], np.cumsum(counts)])
    core_chunks = []
    for s in range(NSPEC):
        idx = order[starts[s]:starts[s + 1]]
        core_chunks.append(np.array_split(idx, NCORES))

    seg_len = []
    for s in range(NSPEC):
        mx = max(len(c) for c in core_chunks[s])
        seg_len.append(int(-(-mx // SEG_ALIGN) * SEG_ALIGN))
    seg_off = np.concatenate([[0], np.cumsum(seg_len)]).astype(int)
    R = int(seg_off[-1])

    xts = []
    for j in range(NCORES):
        xt = np.zeros((AEV_PAD, R), BF16NP)
        xt[AEV] = 1.0  # ones-row for the folded layer-1 bias
        for s in range(NSPEC):
            idx = core_chunks[s][j]
            if len(idx):
                xt[0:AEV, seg_off[s]:seg_off[s] + len(idx)] = \
                    aev[idx].astype(BF16NP).T
        xts.append(xt)

    packed = _pack_weights(params)
    nc = _build_graph(R, seg_off, seg_len, packed)

    in_maps = [{"xt": xts[j], "w": packed["Wp"]} for j in range(NCORES)]

    import os
    trace = bool(int(os.environ.get("KERNEL_TRACE", "0")))
    if trace:
        trace = _ensure_ntff_hook()
    try:
        res = run_bass_kernel_spmd(nc, in_maps, core_ids=list(range(NCORES)),
                                   trace=trace)
    except Exception:
        # A wedged exec unit (rare, transient) is recoverable via axon_reset;
        # retry once without tracing.
        import traceback
        traceback.print_exc()
        try:
            import ctypes
            import jax
            jax.devices()
            lib = ctypes.CDLL("/opt/axon/libaxon_pjrt.so")
            if hasattr(lib, "axon_reset"):
                lib.axon_reset.restype = ctypes.c_int64
                lib.axon_reset()
        except Exception:
            traceback.print_exc()
        res = run_bass_kernel_spmd(nc, in_maps, core_ids=list(range(NCORES)),
                                   trace=False)
    LAST_EXEC_NS = res.exec_time_ns
    LAST_TRACE = getattr(res, "instructions_and_trace", None)
    LAST_RESULTS = res.results

    out = np.zeros(n_atoms, np.float32)
    for j in range(NCORES):
        oj = np.asarray(res.results[j]["out"], np.float32).ravel()
        for s in range(NSPEC):
            idx = core_chunks[s][j]
            if len(idx):
                out[idx] = oj[seg_off[s]:seg_off[s] + len(idx)]
    return out


# revision 60
# speedup vs baseline: 1.0179x; 1.0179x over previous
"""ANI2x-core MoE routing kernel for 8 Trainium2 NeuronCores.

Strategy:
  - Host routes atoms by species (stable argsort) so each expert MLP runs only
    on its own atoms (7x less compute than the masked reference).
  - Data-parallel across 8 cores: each species' atoms are split evenly over the
    cores; every core holds all 7 expert weight sets.
  - aev is pre-transposed on host to [1024, R] (feature-major, zero padded from
    1008, with a constant-1 row at 1008 for the layer-1 bias) and cast to bf16
    so TensorEngine consumes [K-partition, atom] tiles with no transpose.
  - All biases are folded into the matmuls: layer 1 via the xt ones-row;
    layers 2/3 via a ones-row in the spare partition of the last K-chunk (or a
    K=1 ones-matmul when the chunk is full); layer 4 via the ScalarE Copy bias.
  - CELU(0.1) is computed exactly via the identity
        10*celu(u) + 1 = min(exp(y), 1) + relu(y),   y = 10*u = psum
    with the x10 / +1 affine folded into the next layer's weights on host.
    Per layer-chunk: ScalarE Exp from PSUM, DVE tensor_scalar min (bf16 4x),
    then one DVE scalar_tensor_tensor max(psum,0)+m that fuses relu and add.
  - 4-stage software pipeline (L1(t) | L2(t-1) | L3(t-2) | L4(t-3), tails
    emitted first) keeps TensorE dense so the HAM clock stays at 2.4 GHz;
    per-chunk single-bank PSUM tiles (6+2 slots) avoid accumulation-group
    stalls; whole-segment K-chunk DMAs give 2.3KB descriptor lines.
"""

import numpy as np
import ml_dtypes
from contextlib import ExitStack

import concourse.bass as bass
import concourse.mybir as mybir
import concourse.tile as tile
from concourse import bacc
from concourse.bass_utils import run_bass_kernel_spmd

BF16NP = ml_dtypes.bfloat16
BF16 = mybir.dt.bfloat16
F32 = mybir.dt.float32

NCORES = 8
AEV = 1008
AEV_PAD = 1024
NSPEC = 7
LAYERS = [
    [256, 192, 160],  # H
    [224, 192, 160],  # C
    [192, 160, 128],  # O
    [192, 160, 128],  # N
    [160, 128, 96],   # S
    [160, 128, 96],   # F
    [160, 128, 96],   # Cl
]
A_MAX = 512          # atoms per supertile
SEG_ALIGN = 8

LAST_EXEC_NS = None
LAST_TRACE = None
LAST_RESULTS = None


def _fchunks(F):
    return [(i, min(128, F - i)) for i in range(0, F, 128)]


def _pad(F):
    """Pad feature widths in (128, 256) up to 256 (zero weight columns).

    Padded activations compute to exactly 1.0 (y=0 -> min(e^0,1)+relu(0)=1),
    so a pad row doubles as the ones-row for the next layer's folded bias."""
    return 256 if 128 < F < 256 else F


def _kinfo(K_real, K_pad):
    """K-chunk plan over the padded contraction dim.

    Returns (chunks, bias_mm, bias_row): chunks = [(lo, kw)]; if bias_row is
    not None the bias lives in weight row bias_row of the last chunk (a pad
    row of the input, which equals 1). bias_mm means an extra K=1 ones-matmul
    carries the bias instead.
    """
    chunks = [(lo, min(128, K_pad - lo)) for lo in range(0, K_pad, 128)]
    if K_pad > K_real:
        return chunks, False, K_real - 128 * (len(chunks) - 1)
    return chunks, True, None


def _pack_weights(params):
    """Pack all transformed weights (+ folded biases) into one bf16 array.

    Layout per species: [W1 k-blocks (8 x F1 cols)] [W2 k-blocks (F2 cols
    each, bias row folded; +1 extra F2-col bias block if bias_mm)] [W3 ...]
    [W4 (n4 cols)].
    """
    cols = []
    meta = []
    for s in range(NSPEC):
        F1, F2, F3 = LAYERS[s]
        P1, P2, P3 = _pad(F1), _pad(F2), _pad(F3)
        k2, bm2, br2 = _kinfo(F1, P1)
        k3, bm3, br3 = _kinfo(F2, P2)
        k4 = [(lo, min(128, P3 - lo)) for lo in range(0, P3, 128)]
        n4 = len(k4)
        c1 = 8 * P1
        c2 = (len(k2) + (1 if bm2 else 0)) * P2
        c3 = (len(k3) + (1 if bm3 else 0)) * P3
        meta.append(dict(P=(P1, P2, P3), k2=k2, bm2=bm2, br2=br2,
                         k3=k3, bm3=bm3, br3=br3, k4=k4, n4=n4,
                         off1=sum(cols), off2=sum(cols) + c1,
                         off3=sum(cols) + c1 + c2,
                         off4=sum(cols) + c1 + c2 + c3,
                         ncols=c1 + c2 + c3 + n4))
        cols.append(c1 + c2 + c3 + n4)

    Wp = np.zeros((128, sum(cols)), BF16NP)
    b4f = []

    for s in range(NSPEC):
        F1, F2, F3 = LAYERS[s]
        m = meta[s]
        P1, P2, P3 = m["P"]
        net = params[s]
        W = [np.asarray(w, np.float32) for w, _ in net]
        B = [np.asarray(b, np.float32) for _, b in net]

        # L1: weights and bias scaled by 10; ones-row at aev row 1008
        W1h = (10.0 * W[0]).astype(BF16NP)
        bh1 = (10.0 * B[0]).astype(BF16NP)
        for k in range(8):
            r0 = k * 128
            rw = min(128, AEV - r0)
            blk = Wp[:, m["off1"] + k * P1: m["off1"] + k * P1 + F1]
            if rw > 0:
                blk[0:rw] = W1h[r0:r0 + rw]
            if r0 <= AEV < r0 + 128:
                blk[AEV - r0] = bh1

        # L2/L3: unscaled bf16 weights; bias = 10*b - colsum(bf16 W),
        # placed in the pad/ones row of the last K chunk (or an extra block)
        for (Wf, Bf, off, Fr, Fp, kl, bm, br) in (
                (W[1], B[1], m["off2"], F2, P2, m["k2"], m["bm2"], m["br2"]),
                (W[2], B[2], m["off3"], F3, P3, m["k3"], m["bm3"], m["br3"])):
            Wh = Wf.astype(BF16NP)
            bh = (10.0 * Bf - Wh.astype(np.float32).sum(axis=0)).astype(BF16NP)
            for j, (lo, kw) in enumerate(kl):
                blk = Wp[:, off + j * Fp: off + j * Fp + Fr]
                n = max(0, min(kw, Wh.shape[0] - lo))
                if n > 0:
                    blk[0:n] = Wh[lo:lo + n]
                if br is not None and j == len(kl) - 1:
                    blk[br] = bh
            if bm:
                Wp[0, off + len(kl) * Fp: off + len(kl) * Fp + Fr] = bh

        # L4: unscaled bf16; bias correction as float (via ScalarE Copy)
        W4h = W[3].astype(BF16NP)
        b4f.append(float(B[3][0] - 0.1 * W4h.astype(np.float32).sum(axis=0)[0]))
        for j, (lo, kw) in enumerate(m["k4"]):
            n = max(0, min(kw, F3 - lo))
            if n > 0:
                Wp[0:n, m["off4"] + j] = W4h[lo:lo + n, 0]

    return dict(Wp=Wp, b4f=b4f, meta=meta)


def _build_graph(R, seg_off, seg_len, packed):
    """Build the SPMD single-core Bass graph (same on all 8 cores)."""
    nc = bacc.Bacc()

    Wp = packed["Wp"]
    meta = packed["meta"]
    b4f = packed["b4f"]

    xtd = nc.declare_dram_parameter("xt", [AEV_PAD, R], BF16, isOutput=False)
    wd = nc.declare_dram_parameter("w", list(Wp.shape), BF16, isOutput=False)
    outd = nc.declare_dram_parameter("out", [1, R], F32, isOutput=True)

    amax = mybir.AluOpType.max
    amin = mybir.AluOpType.min
    add = mybir.AluOpType.add
    Exp = mybir.ActivationFunctionType.Exp
    Copy = mybir.ActivationFunctionType.Copy

    Lmax = max(seg_len)

    with ExitStack() as ctx:
        tc = ctx.enter_context(tile.TileContext(nc))
        wpool = ctx.enter_context(tc.tile_pool(name="weights", bufs=1))
        xpool = ctx.enter_context(tc.tile_pool(name="x", bufs=16))
        hpool = ctx.enter_context(tc.tile_pool(name="h", bufs=10))
        epool = ctx.enter_context(tc.tile_pool(name="er", bufs=10))
        ypool = ctx.enter_context(tc.tile_pool(name="y", bufs=3))
        pspool = ctx.enter_context(tc.tile_pool(name="ps", bufs=6, space="PSUM"))
        ps4pool = ctx.enter_context(tc.tile_pool(name="ps4", bufs=2, space="PSUM"))

        ones = wpool.tile([1, Lmax], BF16, tag="ones")
        nc.gpsimd.memset(ones, 1.0)

        def epi_re(ps, fw, A):
            """exp + clamp from one chunk's psum: m = min(exp(ps), 1)."""
            e = epool.tile([128, A_MAX], BF16, tag="e")
            m = epool.tile([128, A_MAX], BF16, tag="m")
            sl = (slice(0, fw), slice(0, A))
            nc.scalar.activation(e[sl], ps[sl], Exp, bias=0.0, scale=1.0)
            nc.vector.tensor_scalar_min(m[sl], e[sl], 1.0)
            return ps, m

        def epi_fuse(pairs, fchn, A, htile):
            for ci, (f0, fw) in enumerate(fchn):
                ps, m = pairs[ci]
                nc.vector.scalar_tensor_tensor(
                    htile[0:fw, ci * A:ci * A + A], ps[0:fw, 0:A], 0.0,
                    m[0:fw, 0:A], amax, add)

        def hidden_layer(hin, klist, bias_mm, woff, Fp, wt, A):
            fch = _fchunks(Fp)
            nmm = len(klist) + (1 if bias_mm else 0)
            pairs = []
            for ci, (f0, fw) in enumerate(fch):
                ps = pspool.tile([128, A_MAX], F32, tag="ps")
                for n0 in range(0, A, 512):
                    nw = min(512, A - n0)
                    mi = 0
                    for j, (lo, kw) in enumerate(klist):
                        nc.tensor.matmul(
                            ps[0:fw, n0:n0 + nw],
                            wt[0:kw, woff + j * Fp + f0: woff + j * Fp + f0 + fw],
                            hin[0:kw, j * A + n0: j * A + n0 + nw],
                            start=(mi == 0), stop=(mi == nmm - 1))
                        mi += 1
                    if bias_mm:
                        nc.tensor.matmul(
                            ps[0:fw, n0:n0 + nw],
                            wt[0:1, woff + len(klist) * Fp + f0:
                               woff + len(klist) * Fp + f0 + fw],
                            ones[0:1, n0:n0 + nw],
                            start=False, stop=True)
                pairs.append(epi_re(ps, fw, A))
            hout = hpool.tile([128, 2 * A_MAX], BF16, tag="h")
            epi_fuse(pairs, fch, A, hout)
            return hout

        def emit_l2(st):
            s, t0, A, h1, wt, y_seg, last = st
            m = meta[s]
            h2 = hidden_layer(h1, m["k2"], m["bm2"], m["off2"] - m["off1"],
                              m["P"][1], wt, A)
            return (s, t0, A, h2, wt, y_seg, last)

        def emit_l3(st):
            s, t0, A, h2, wt, y_seg, last = st
            m = meta[s]
            h3 = hidden_layer(h2, m["k3"], m["bm3"], m["off3"] - m["off1"],
                              m["P"][2], wt, A)
            return (s, t0, A, h3, wt, y_seg, last)

        def emit_l4(st):
            s, t0, A, h3, wt, y_seg, last = st
            m = meta[s]
            kch4 = m["k4"]
            ps4 = ps4pool.tile([1, A_MAX], F32, tag="ps4")
            for n0 in range(0, A, 512):
                nw = min(512, A - n0)
                for j, (lo, kw) in enumerate(kch4):
                    nc.tensor.matmul(
                        ps4[0:1, n0:n0 + nw],
                        wt[0:kw, m["off4"] - m["off1"] + j: m["off4"] - m["off1"] + j + 1],
                        h3[0:kw, j * A + n0: j * A + n0 + nw],
                        start=(j == 0), stop=(j == len(kch4) - 1))
            nc.scalar.activation(y_seg[0:1, t0:t0 + A], ps4[0:1, 0:A], Copy,
                                 bias=b4f[s], scale=0.1)
            if last:
                L = seg_len[s]
                nc.sync.dma_start(out=outd[0:1, seg_off[s]:seg_off[s] + L],
                                  in_=y_seg[0:1, 0:L])

        def emit_l1(s, wt, xk, y_seg, t0, L, A):
            P1 = meta[s]["P"][0]
            fch1 = _fchunks(P1)
            pairs = []
            for ci, (f0, fw) in enumerate(fch1):
                ps = pspool.tile([128, A_MAX], F32, tag="ps")
                for n0 in range(0, A, 512):
                    nw = min(512, A - n0)
                    for k in range(8):
                        nc.tensor.matmul(
                            ps[0:fw, n0:n0 + nw],
                            wt[:, k * P1 + f0: k * P1 + f0 + fw],
                            xk[k][:, t0 + n0: t0 + n0 + nw],
                            start=(k == 0), stop=(k == 7))
                pairs.append(epi_re(ps, fw, A))
            h1 = hpool.tile([128, 2 * A_MAX], BF16, tag="h")
            epi_fuse(pairs, fch1, A, h1)
            return (s, t0, A, h1, wt, y_seg, t0 + A >= L)

        # 3-stage software pipeline over all supertiles, tails emitted first
        work = []   # (s, t0, A) in order
        for s in range(NSPEC):
            t0 = 0
            first = A_MAX
            while t0 < seg_len[s]:
                A = min(first if t0 == 0 else A_MAX, seg_len[s] - t0)
                work.append((s, t0, A))
                t0 += A

        p1 = p2 = p3 = None   # stage outputs awaiting next stage
        cur_s = -1
        wt = xk = y_seg = None
        for (s, t0, Awk) in work:
            if s != cur_s:
                # new species: segment DMAs (first supertile's columns first)
                m = meta[s]
                L = seg_len[s]
                c0 = min(A_MAX, L) if s == 0 else L
                wt = wpool.tile([128, m["ncols"]], BF16, tag=f"w_{s}")
                if s == 0:
                    # L1 weight block gates the first matmul: lead the scalar ring
                    c1w = 8 * meta[0]["P"][0]
                    nc.scalar.dma_start(out=wt[:, 0:c1w],
                                        in_=wd[:, m["off1"]:m["off1"] + c1w])
                xk = []
                for k in range(8):
                    xt_k = xpool.tile([128, Lmax], BF16, tag="xk")
                    eng = nc.sync if (s > 0 or k % 2 == 0) else nc.scalar
                    eng.dma_start(
                        out=xt_k[:, 0:c0],
                        in_=xtd[k * 128:(k + 1) * 128, seg_off[s]:seg_off[s] + c0])
                    xk.append(xt_k)
                if s == 0:
                    c1w = 8 * meta[0]["P"][0]
                    nc.sync.dma_start(
                        out=wt[:, c1w:m["ncols"]],
                        in_=wd[:, m["off1"] + c1w:m["off1"] + m["ncols"]])
                else:
                    nc.sync.dma_start(out=wt,
                                      in_=wd[:, m["off1"]:m["off1"] + m["ncols"]])
                if c0 < L:
                    for k in range(8):
                        nc.sync.dma_start(
                            out=xk[k][:, c0:L],
                            in_=xtd[k * 128:(k + 1) * 128,
                                    seg_off[s] + c0:seg_off[s] + L])
                y_seg = ypool.tile([1, Lmax], F32, tag="y")
                cur_s = s
            if p3 is not None:
                emit_l4(p3)
            p3 = emit_l3(p2) if p2 is not None else None
            p2 = emit_l2(p1) if p1 is not None else None
            p1 = emit_l1(s, wt, xk, y_seg, t0, seg_len[s], Awk)
        if p3 is not None:
            emit_l4(p3)
        if p2 is not None:
            emit_l4(emit_l3(p2))
        if p1 is not None:
            emit_l4(emit_l3(emit_l2(p1)))

    nc.finalize()
    return nc


def _ensure_ntff_hook():
    """Provide antenv.axon_hooks (missing in this image) so trace=True works."""
    try:
        from antenv.axon_hooks import get_axon_ntff_profile_hook
        return get_axon_ntff_profile_hook() is not None
    except ImportError:
        pass
    try:
        import types, sys
        import antenv
        from trn_agent_boot.trn_boot import _ntff_profile_via_ctypes
        hook = _ntff_profile_via_ctypes("/opt/axon/libaxon_pjrt.so")
        if hook is None:
            return False
        mod = types.ModuleType("antenv.axon_hooks")
        state = {"hook": hook}
        mod.get_axon_ntff_profile_hook = lambda: state["hook"]
        mod.set_axon_ntff_profile_hook = lambda h: state.__setitem__("hook", h)
        sys.modules["antenv.axon_hooks"] = mod
        antenv.axon_hooks = mod
        import concourse.bass_utils as bu
        bu.upload_artifacts = lambda tmpdir: f"local://{tmpdir}"
        return True
    except Exception:
        import traceback
        traceback.print_exc()
        return False


def kernel(species, aev, params):
    global LAST_EXEC_NS, LAST_TRACE, LAST_RESULTS
    species = np.asarray(species, np.int32)
    aev = np.asarray(aev, np.float32)
    n_atoms = species.shape[0]

    order = np.argsort(species, kind="stable")
    counts = np.bincount(species, minlength=NSPEC)

    starts = np.concatenate([[0], np.cumsum(counts)])
    core_chunks = []
    for s in range(NSPEC):
        idx = order[starts[s]:starts[s + 1]]
        core_chunks.append(np.array_split(idx, NCORES))

    seg_len = []
    for s in range(NSPEC):
        mx = max(len(c) for c in core_chunks[s])
        seg_len.append(int(-(-mx // SEG_ALIGN) * SEG_ALIGN))
    seg_off = np.concatenate([[0], np.cumsum(seg_len)]).astype(int)
    R = int(seg_off[-1])

    xts = []
    for j in range(NCORES):
        xt = np.zeros((AEV_PAD, R), BF16NP)
        xt[AEV] = 1.0  # ones-row for the folded layer-1 bias
        for s in range(NSPEC):
            idx = core_chunks[s][j]
            if len(idx):
                xt[0:AEV, seg_off[s]:seg_off[s] + len(idx)] = \
                    aev[idx].astype(BF16NP).T
        xts.append(xt)

    packed = _pack_weights(params)
    nc = _build_graph(R, seg_off, seg_len, packed)

    in_maps = [{"xt": xts[j], "w": packed["Wp"]} for j in range(NCORES)]

    import os
    trace = bool(int(os.environ.get("KERNEL_TRACE", "0")))
    if trace:
        trace = _ensure_ntff_hook()
    try:
        res = run_bass_kernel_spmd(nc, in_maps, core_ids=list(range(NCORES)),
                                   trace=trace)
    except Exception:
        # A wedged exec unit (rare, transient) is recoverable via axon_reset;
        # retry once without tracing.
        import traceback
        traceback.print_exc()
        try:
            import ctypes
            import jax
            jax.devices()
            lib = ctypes.CDLL("/opt/axon/libaxon_pjrt.so")
            if hasattr(lib, "axon_reset"):
                lib.axon_reset.restype = ctypes.c_int64
                lib.axon_reset()
        except Exception:
            traceback.print_exc()
        res = run_bass_kernel_spmd(nc, in_maps, core_ids=list(range(NCORES)),
                                   trace=False)
    LAST_EXEC_NS = res.exec_time_ns
    LAST_TRACE = getattr(res, "instructions_and_trace", None)
    LAST_RESULTS = res.results

    out = np.zeros(n_atoms, np.float32)
    for j in range(NCORES):
        oj = np.asarray(res.results[j]["out"], np.float32).ravel()
        for s in range(NSPEC):
            idx = core_chunks[s][j]
            if len(idx):
                out[idx] = oj[seg_off[s]:seg_off[s] + len(idx)]
    return out
